# revision 39
# baseline (speedup 1.0000x reference)
"""Trainium2 Bass kernel for CapsNet dynamic routing (ClassCapsules).

Reference computation (B=256, R=1152, C=10, O=16, I=8, 3 routing iters):
    u_hat[b,r,c,o] = sum_i W[r,c,o,i] * x[b,r,i]
    b_ij = 0
    for it in 3:
        c_ij = softmax(b_ij, axis=1)                      # over c
        s = sum_r c_ij[r,c] * u_hat[b,r,c,o] + bias       # [B,C,O]
        v = squash(s)
        if it < 2:
            b_ij += mean_b sum_o u_hat[b,r,c,o] v[b,c,o]  # [R,C]
    return v[..., None]

u_hat ([B,R,C,O] = 189MB fp32) is never materialized.  Both routing
contractions are re-associated through the factorization
    s[b,co]    = x~[b,(ri)] @ (c∘W~)[(ri),(co)]
    agree[r,c] = sum_{i,o} W~[(ri),(co)] * G[(ri),(co)],
                 G = (1/B) x~^T v
with x~ = x viewed as [B, R*I] and W~ = W viewed as [R*I, C*O].

Distribution: R is sharded 8 ways (144 r's per core).  Per iteration the
partial s ([256,160], f16 on the wire) is summed across cores with one
AllReduce; the last iteration uses a ReduceScatter instead and each core
squashes + outputs its own 32-batch shard.  agree/b_ij/c_ij are fully
local to each core's r-shard.

Matmul inputs (x~, W~, c∘W~, v) are bf16: fp32 matmuls on TRN2 pay ~4x
in PE streaming and get no fast weight load.  PSUM accumulation stays
fp32 and the softmax/squash chain stays fp32, so only u_hat-level
rounding (~4e-3 relative) enters the routing.
"""

import os
import sys
import types

sys.path.insert(0, "/opt/trn_rl_repo")

# Shim antenv.axon_hooks (absent on this image) so BASS_TRACE=1 profiling
# works through run_bass_kernel_spmd's axon path.  Harmless when unused.
try:
    import antenv.axon_hooks  # noqa: F401
except ImportError:
    try:
        _hooks = types.ModuleType("antenv.axon_hooks")
        _hooks._hook = None
        _hooks.set_axon_ntff_profile_hook = lambda h: setattr(_hooks, "_hook", h)
        _hooks.get_axon_ntff_profile_hook = lambda: _hooks._hook
        sys.modules["antenv.axon_hooks"] = _hooks
        import antenv
        antenv.axon_hooks = _hooks
        from trn_agent_boot.trn_boot import _ntff_profile_via_ctypes
        _hooks.set_axon_ntff_profile_hook(
            _ntff_profile_via_ctypes("/opt/axon/libaxon_pjrt.so")
        )
    except Exception:
        pass

import numpy as np
import ml_dtypes

import concourse.bacc as bacc
import concourse.bass as bass
import concourse.tile as tile
from concourse import library_config, mybir
import concourse.bass_utils as _bass_utils
from concourse.bass_utils import run_bass_kernel_spmd

if os.environ.get("BASS_TRACE"):
    _bass_utils.upload_artifacts = lambda tmpdir: ""  # no bucket access here

LAST_RESULT = None

F32 = mybir.dt.float32
F16 = mybir.dt.float16
BF16 = mybir.dt.bfloat16
ALU = mybir.AluOpType
ACT = mybir.ActivationFunctionType

B, R, C, O, I = 256, 1152, 10, 16, 8
CO = C * O                      # 160
N_CORES = 8
R_LOC = R // N_CORES            # 144
RI_LOC = R_LOC * I              # 1152
NG = RI_LOC // 128              # 9 groups of 128 (r,i) rows
NB = B // 128                   # 2 batch partition chunks
B_SHARD = B // N_CORES          # 32 batches output per core
ITERS = 3
RPG = 128 // I                  # 16 r's per group

# With CC exchanges the stream-init barrier attaches to the first real
# AllReduce; a separate warm-up AR only adds serialized CC-stream time.
# (It is required for K_RDMA=1: PJRT staggers collective-free NEFF
# launches by milliseconds.)
WARM_AR = os.environ.get(
    "K_WARM_AR", "1" if os.environ.get("K_RDMA", "0") == "1" else "0"
) == "1"
CC_F16 = os.environ.get("K_CC_F16", "1") == "1"
CC_DT = F16 if CC_F16 else F32
MM_BF16 = os.environ.get("K_BF16", "1") == "1"
MM_DT = BF16 if MM_BF16 else F32
MM_NP = ml_dtypes.bfloat16 if MM_BF16 else np.float32
N_WARM_MM = int(os.environ.get("K_WARM_MM", "24"))
WARM_N = int(os.environ.get("K_WARM_N", "512"))
N_WARM_B = int(os.environ.get("K_WARM_B", "18"))
RDMA = os.environ.get("K_RDMA", "0") == "1"
GPB = int(os.environ.get("K_GPB", "3"))      # psum_g bufs


def _squash(nc, eps_sb, t, n_part, nb, pool, out_dt=F32):
    """v = t * n2/((1+n2)*sqrt(n2+eps)); t: [n_part, nb, CO], reduce over o."""
    nc_ = nb * C
    tf = t.rearrange("p nb co -> p (nb co)")
    sq = pool.tile([n_part, nb * CO], F32, tag="sq")
    nc.vector.tensor_mul(sq, tf, tf)
    n2 = pool.tile([n_part, nc_], F32, tag="n2")
    nc.vector.reduce_sum(
        n2, sq.rearrange("p (nb c o) -> p nb c o", nb=nb, c=C),
        axis=mybir.AxisListType.X,
    )
    rt = pool.tile([n_part, nc_], F32, tag="rt")
    nc.scalar.activation(rt, n2, ACT.Sqrt, bias=eps_sb[:n_part])
    den = pool.tile([n_part, nc_], F32, tag="den")
    nc.vector.scalar_tensor_tensor(
        out=den, in0=n2, scalar=1.0, in1=rt, op0=ALU.add, op1=ALU.mult,
    )
    rec = pool.tile([n_part, nc_], F32, tag="rec")
    nc.vector.reciprocal(rec, den)
    fac = pool.tile([n_part, nc_], F32, tag="fac")
    nc.vector.tensor_mul(fac, n2, rec)
    v = pool.tile([n_part, nb, CO], out_dt, tag="v")
    fac_b = fac.rearrange(
        "p (nb c one) -> p nb c one", nb=nb, c=C
    ).broadcast_to([n_part, nb, C, O])
    nc.vector.tensor_tensor(
        out=v.rearrange("p nb (c o) -> p nb c o", c=C),
        in0=t.rearrange("p nb (c o) -> p nb c o", c=C),
        in1=fac_b,
        op=ALU.mult,
    )
    return v


def build():
    nc = bacc.Bacc("TRN2", target_bir_lowering=False, debug=False,
                   num_devices=N_CORES)

    xt_d = nc.dram_tensor("xt", [RI_LOC, B], MM_DT, kind="ExternalInput")
    xb_d = nc.dram_tensor("xb", [B, RI_LOC], MM_DT, kind="ExternalInput")
    wg_d = nc.dram_tensor("wg", [RI_LOC, CO], MM_DT, kind="ExternalInput")
    bias_d = nc.dram_tensor("biasf", [CO], F32, kind="ExternalInput")
    sel_d = nc.dram_tensor("sel", [128, RPG], F32, kind="ExternalInput")
    selT_d = nc.dram_tensor("selT", [RPG, 128], F32, kind="ExternalInput")
    if RDMA:
        # every core outputs the full squashed v; the host slices
        y_d = nc.dram_tensor("y", [128, NB * CO], F32, kind="ExternalOutput")
    else:
        y_d = nc.dram_tensor("y", [B_SHARD, CO], F32, kind="ExternalOutput")

    rg = [list(range(N_CORES))]

    with tile.TileContext(nc) as tc:
        with (
            tc.tile_pool(name="singles", bufs=1) as singles,
            tc.tile_pool(name="cw_pool", bufs=2) as cw_pool,
            tc.tile_pool(name="work", bufs=2) as work,
            tc.tile_pool(name="small", bufs=3) as small,
            tc.tile_pool(name="psum_s", bufs=1, space="PSUM") as psum_s,
            tc.tile_pool(name="psum_g", bufs=GPB, space="PSUM") as psum_g,
            tc.tile_pool(name="psum_misc", bufs=1, space="PSUM") as psum_misc,
            tc.tile_pool(name="dram", bufs=2, space="DRAM") as dram,
        ):
            rdma_waits = []
            if RDMA:
                nc.gpsimd.load_library(library_config.remote_dma)
                rsems = [nc.alloc_semaphore(f"rsem{e}") for e in range(ITERS)]
                lsem = nc.alloc_semaphore("lsem")
                # recv[e][:, d, :] is written by the peer whose (rid, tpb)
                # XOR-differs by d; slot 0 is the local partial (also the
                # broadcast source).  One buffer per exchange, never reused.
                recvs = [
                    singles.tile([128, N_CORES, NB * CO], CC_DT,
                                 name=f"recv{e}")
                    for e in range(ITERS)
                ]

            if WARM_AR:
                warm_sb = singles.tile([1, 8], F32)
                nc.vector.memset(warm_sb, 0.0)
                warm_in = dram.tile([8], F32)
                warm_out = dram.tile([8], F32)
                nc.gpsimd.dma_start(out=warm_in[:], in_=warm_sb[0, :])
                nc.gpsimd.collective_compute(
                    "AllReduce", ALU.add, replica_groups=rg,
                    ins=[warm_in[:]], outs=[warm_out[:]],
                )

            # ---- load inputs ----
            XT = singles.tile([128, NG, B], MM_DT)     # x~ [(ri),b] chunked
            nc.sync.dma_start(
                out=XT, in_=xt_d.ap().rearrange("(g p) b -> p g b", p=128)
            )
            XB = []                                    # x [b,(ri)] 2 p-chunks
            for kb in range(NB):
                t = singles.tile([128, RI_LOC], MM_DT, tag=f"xb{kb}",
                                 name=f"xb_sb{kb}")
                nc.sync.dma_start(out=t, in_=xb_d[kb * 128:(kb + 1) * 128, :])
                XB.append(t)
            WG = singles.tile([128, NG, CO], MM_DT)    # W~ [(ri),(co)] chunked
            nc.sync.dma_start(
                out=WG, in_=wg_d.ap().rearrange("(g p) n -> p g n", p=128)
            )
            biasb = singles.tile([128, CO], F32)
            nc.sync.dma_start(
                out=biasb,
                in_=bass.AP(tensor=bias_d, offset=0, ap=[[0, 128], [1, CO]]),
            )
            sel_sb = singles.tile([128, RPG], F32)
            nc.sync.dma_start(out=sel_sb, in_=sel_d[:, :])
            selT_sb = singles.tile([RPG, 128], F32)
            nc.sync.dma_start(out=selT_sb, in_=selT_d[:, :])
            if MM_BF16:
                sel_mm = singles.tile([128, RPG], MM_DT, name="sel_mm")
                nc.scalar.copy(sel_mm, sel_sb)
            else:
                sel_mm = sel_sb
            sel_cc = singles.tile([128, RPG], CC_DT, name="sel_cc")
            nc.scalar.copy(sel_cc, sel_sb)

            eps_sb = singles.tile([128, 1], F32)
            nc.vector.memset(eps_sb, 1e-8)

            esr = None   # [16, 90]: running exp(b_ij)

            for it in range(ITERS):
                # ---- CW = c∘W~ (it>0); it=0 uses uniform c=0.1 folded later
                if it == 0:
                    CW = WG
                else:
                    # cnorm[16, 90] = esr * (1/sum_c esr), broadcast to the
                    # 128 (r,i) partition rows via the selT matmul.
                    cp_ps = psum_misc.tile([128, NG * C], F32, tag="cp",
                                           name=f"cp_ps_{it}")
                    nc.tensor.matmul(cp_ps, selT_sb, esr, start=True,
                                     stop=True)
                    cp_sb = small.tile([128, NG * C], F32, tag="cpart",
                                       name=f"cp_sb_{it}")
                    nc.scalar.copy(cp_sb, cp_ps)
                    CW = cw_pool.tile([128, NG, CO], MM_DT, tag="cw",
                                      name=f"cw_{it}")
                    c_b = cp_sb.rearrange(
                        "p (g c one) -> p g c one", g=NG, one=1
                    ).broadcast_to([128, NG, C, O])
                    nc.vector.tensor_tensor(
                        out=CW.rearrange("p g (c o) -> p g c o", c=C),
                        in0=WG.rearrange("p g (c o) -> p g c o", c=C),
                        in1=c_b, op=ALU.mult,
                    )

                # ---- s partial: [256,160] = x~^T @ CW, K = (ri) local ----
                s_ps = [psum_s.tile([128, CO], F32, tag=f"s{kb}",
                                    name=f"s_ps{kb}_{it}")
                        for kb in range(NB)]
                for kb in range(NB):
                    for g in range(NG):
                        nc.tensor.matmul(
                            s_ps[kb],
                            XT[:, g, kb * 128:(kb + 1) * 128],
                            CW[:, g, :],
                            start=(g == 0),
                            stop=(g == NG - 1),
                        )

                if RDMA:
                    recv = recvs[it]
                    rsem = rsems[it]
                    cp_insts = []
                    for kb in range(NB):
                        cp_insts.append(nc.scalar.copy(
                            recv[:, 0, kb * CO:(kb + 1) * CO], s_ps[kb]
                        ))
                    for dd in range(1, N_CORES):
                        rd: list = [None] * N_CORES
                        rd[dd] = (0, dd)
                        nc.gpsimd.remote_dma_broadcast(
                            out_ap=recv[:, dd, :],
                            in_ap=recv[:, 0, :],
                            remote_sem=rsem,
                            local_sem=lsem,
                            rdests=rd,
                        )
                    nc.gpsimd.trigger_dma(count=None)
                    if N_WARM_MM and it < ITERS - 1:
                        warm_ps = psum_misc.tile([RPG, 512], F32,
                                                 tag="warmps",
                                                 name=f"warm_ps_{it}")
                        warm_rhs = XT.rearrange("p g b -> p (g b)")[:, :WARM_N]
                        for wi in range(N_WARM_MM):
                            nc.tensor.matmul(
                                warm_ps[:, :WARM_N], sel_mm, warm_rhs,
                                start=(wi == 0), stop=True,
                                skip_group_check=True,
                            )
                    # tree-reduce the 8 slots; the first add must wait for
                    # the 7 remote arrivals (2 sem increments each).  The
                    # wait is injected post-compile (see build() tail): the
                    # single-core scheduling sim can never see remote sem
                    # updates and would report a deadlock.
                    # The arrival gate (wait rsem>=14) is spliced in front of
                    # this reduce post-compile: the single-core scheduling
                    # sim cannot see remote sem updates and would deadlock,
                    # and compute instructions carry at most one wait slot.
                    red = work.tile([128, NB, CO], F32, tag="red",
                                    name=f"red_{it}")
                    red_i = nc.vector.reduce_sum(
                        red.rearrange("p nb co -> p (nb co)"),
                        recv.rearrange("p s x -> p x s"),
                        axis=mybir.AxisListType.X,
                    )
                    rdma_waits.append((red_i.ins.name, rsem))
                    t = work.tile([128, NB, CO], F32, tag="t",
                                  name=f"t_{it}")
                    bias_b = biasb.rearrange(
                        "p (one co) -> p one co", one=1
                    ).broadcast_to([128, NB, CO])
                    nc.vector.scalar_tensor_tensor(
                        out=t, in0=red,
                        scalar=(0.1 if it == 0 else 1.0),
                        in1=bias_b, op0=ALU.mult, op1=ALU.add,
                    )
                    if it < ITERS - 1:
                        v_sb = _squash(nc, eps_sb, t, 128, NB, work,
                                       out_dt=MM_DT)
                    else:
                        v_out = _squash(nc, eps_sb, t, 128, NB, work)
                        nc.sync.dma_start(
                            out=y_d[:, :],
                            in_=v_out.rearrange("p nb co -> p (nb co)"),
                        )
                else:
                    cc_in = dram.tile([NB, 128, CO], CC_DT, tag="cc_in",
                                      name=f"cc_in_{it}")
                    for kb in range(NB):
                        s_stage = work.tile([128, CO], CC_DT,
                                            tag=f"sstage{kb}",
                                            name=f"s_stage{kb}_{it}")
                        nc.scalar.copy(s_stage, s_ps[kb])
                        nc.sync.dma_start(out=cc_in[kb, :, :], in_=s_stage)
                        if kb == NB - 1:
                            dsq = small.tile([1, 1], F32, tag="dsq",
                                             name=f"dsq_{it}")
                            nc.scalar.activation(dsq, s_stage[:1, :1],
                                                 ACT.Sqrt,
                                                 bias=eps_sb[:1], scale=0.0)

                if not RDMA and it < ITERS - 1:
                    # ---- AllReduce s; every core squashes the full batch
                    cc_out = dram.tile([NB, 128, CO], CC_DT, tag="cc_out",
                                       name=f"cc_out_{it}")
                    nc.gpsimd.collective_compute(
                        "AllReduce", ALU.add, replica_groups=rg,
                        ins=[cc_in.opt()], outs=[cc_out.opt()],
                    )
                    s_sb = work.tile([128, NB, CO], CC_DT, tag="ssb",
                                     name=f"s_sb_{it}")
                    for kb in range(NB):
                        nc.sync.dma_start(
                            out=s_sb[:, kb, :], in_=cc_out[kb, :, :]
                        )
                    if N_WARM_MM:
                        # Phase A: dependency-free burst keeps the PE's HAM
                        # window busy through the AllReduce wait.
                        warm_ps = psum_misc.tile([RPG, 512], F32,
                                                 tag="warmps",
                                                 name=f"warm_ps_{it}")
                        warm_rhs = XT.rearrange(
                            "p g b -> p (g b)")[:, :WARM_N]
                        for wi in range(N_WARM_MM):
                            nc.tensor.matmul(
                                warm_ps[:, :WARM_N], sel_mm, warm_rhs,
                                start=(wi == 0), stop=True,
                                skip_group_check=True,
                            )
                        # Phase B: keyed on the AllReduce result, so it
                        # spans the squash chain and hands the G matmuls a
                        # warm (2.4 GHz) PE.  Output is discarded.
                        warm_b = s_sb.rearrange("p nb co -> p (nb co)")
                        for wi in range(N_WARM_B):
                            nc.tensor.matmul(
                                warm_ps[:, :NB * CO], sel_cc, warm_b,
                                start=(wi == 0), stop=True,
                                skip_group_check=True,
                            )
                    t = work.tile([128, NB, CO], F32, tag="t",
                                  name=f"t_{it}")
                    bias_b = biasb.rearrange(
                        "p (one co) -> p one co", one=1
                    ).broadcast_to([128, NB, CO])
                    nc.vector.scalar_tensor_tensor(
                        out=t, in0=s_sb,
                        scalar=(0.1 if it == 0 else 1.0),
                        in1=bias_b, op0=ALU.mult, op1=ALU.add,
                    )
                    v_sb = _squash(nc, eps_sb, t, 128, NB, work,
                                   out_dt=MM_DT)

                if it < ITERS - 1:
                    # ---- G = (1/B) x~^T v ; agree = sum_io W∘G ----
                    # PSUM banks are drained by the scalar engine (idle
                    # here), freeing the DVE for the squash and keeping the
                    # PE from stalling on bank recycling; the W∘G multiply
                    # then runs as one big stt.
                    Q_all = small.tile([128, NG * C], F32, tag="qall",
                                       name=f"qall_{it}")
                    g_all = work.tile([128, NG, CO], F32, tag="gall",
                                      name=f"gall_{it}")
                    p9 = work.tile([128, NG, CO], F32, tag="p9",
                                   name=f"p9_{it}")
                    for g in range(NG):
                        g_ps = psum_g.tile([128, CO], F32, tag="gps",
                                           name=f"g_ps_{it}_{g}")
                        for kb in range(NB):
                            nc.tensor.matmul(
                                g_ps,
                                XB[kb][:, g * 128:(g + 1) * 128],
                                v_sb[:, kb, :],
                                start=(kb == 0),
                                stop=(kb == NB - 1),
                            )
                        nc.scalar.copy(g_all[:, g, :], g_ps)
                    nc.vector.scalar_tensor_tensor(
                        out=p9, in0=g_all, scalar=1.0 / B,
                        in1=WG, op0=ALU.mult, op1=ALU.mult,
                    )
                    for lo, hi in ((0, 4), (4, 8), (8, 9)):
                        nc.vector.reduce_sum(
                            Q_all[:, lo * C:hi * C],
                            p9[:, lo:hi, :].rearrange(
                                "p g (c o) -> p (g c) o", c=C),
                            axis=mybir.AxisListType.X,
                        )
                    agree_ps = psum_misc.tile([RPG, NG * C], F32, tag="agree",
                                              name=f"agree_{it}")
                    nc.tensor.matmul(agree_ps, sel_sb, Q_all,
                                     start=True, stop=True)

                    # ---- softmax logits, chained multiplicatively:
                    # softmax(b_prev + agree) ∝ c_prev_normalized * exp(agree)
                    # (per-row scale from early normalization cancels) ----
                    eexp = small.tile([RPG, NG * C], F32, tag="eexp",
                                      name=f"eexp_{it}")
                    nc.scalar.activation(eexp, agree_ps, ACT.Exp)
                    if it == 0:
                        base = eexp
                    else:
                        base = small.tile([RPG, NG * C], F32, tag="esrr",
                                          name=f"base_{it}")
                        nc.vector.tensor_mul(base, esr, eexp)
                    den = small.tile([RPG, NG], F32, tag="sden",
                                     name=f"den_{it}")
                    nc.vector.reduce_sum(
                        den,
                        base.rearrange("p (g c) -> p g c", g=NG),
                        axis=mybir.AxisListType.X,
                    )
                    rec_r = small.tile([RPG, NG], F32, tag="srec",
                                       name=f"rec_{it}")
                    nc.vector.reciprocal(rec_r, den)
                    esr_n = small.tile([RPG, NG * C], F32, tag="esr",
                                       name=f"esr_{it}")
                    rec_b = rec_r.rearrange(
                        "p (g one) -> p g one", one=1
                    ).broadcast_to([RPG, NG, C])
                    nc.vector.tensor_tensor(
                        out=esr_n.rearrange("p (g c) -> p g c", g=NG),
                        in0=base.rearrange("p (g c) -> p g c", g=NG),
                        in1=rec_b, op=ALU.mult,
                    )
                    esr = esr_n       # normalized c_ij rows, [16, 90]
                elif not RDMA:
                    # ---- final iter: ReduceScatter; squash own b-shard ----
                    rs_out = dram.tile([B_SHARD * CO], CC_DT, tag="rs_out")
                    nc.gpsimd.collective_compute(
                        "ReduceScatter", ALU.add, replica_groups=rg,
                        ins=[cc_in.opt()], outs=[rs_out[:]],
                    )
                    s_sb = work.tile([B_SHARD, 1, CO], CC_DT, tag="fs")
                    nc.sync.dma_start(
                        out=s_sb,
                        in_=rs_out.rearrange("(p one n) -> p one n",
                                             n=CO, one=1),
                    )
                    t = work.tile([B_SHARD, 1, CO], F32, tag="ft")
                    bias_b1 = biasb[:B_SHARD, :].rearrange(
                        "p (one co) -> p one co", one=1
                    )
                    nc.vector.scalar_tensor_tensor(
                        out=t, in0=s_sb, scalar=1.0,
                        in1=bias_b1, op0=ALU.mult, op1=ALU.add,
                    )
                    v = _squash(nc, eps_sb, t, B_SHARD, 1, work)
                    nc.sync.dma_start(
                        out=y_d[:, :], in_=v.rearrange("p one co -> p (one co)")
                    )

    nc.compile()
    # Splice the remote-arrival gates in front of each slot reduce, now
    # that scheduling is done.  A standalone EventSemaphore on the DVE
    # queue stalls it (in-order) until the 7 remote transfers landed.
    f = nc.m.functions[0]
    for r4_name, sem in rdma_waits:
        w = nc.vector.wait_ge(sem, 14)
        w_ins = None
        for b in f.blocks:
            if b.instructions and b.instructions[-1] is w.ins:
                w_ins = b.instructions.pop()
                break
        assert w_ins is not None, "gate wait not found at a block tail"
        placed = False
        for b in f.blocks:
            for idx, i in enumerate(b.instructions):
                if i.name == r4_name:
                    b.instructions.insert(idx, w_ins)
                    placed = True
                    break
            if placed:
                break
        assert placed, f"reduce {r4_name} not found for gate splice"
    return nc


_NC = None


def kernel(x: np.ndarray, W: np.ndarray, bias: np.ndarray) -> np.ndarray:
    global _NC
    if _NC is None:
        _NC = build()

    x = np.ascontiguousarray(x, dtype=np.float32)
    W = np.ascontiguousarray(W, dtype=np.float32)
    bias = np.ascontiguousarray(bias, dtype=np.float32)

    biasf = bias.reshape(CO)
    sel = np.zeros((128, RPG), dtype=np.float32)
    sel[np.arange(128), np.arange(128) // I] = 1.0
    selT = np.ascontiguousarray(sel.T)

    in_maps = []
    for k in range(N_CORES):
        r0, r1 = k * R_LOC, (k + 1) * R_LOC
        xk = x[:, r0:r1, :].reshape(B, RI_LOC)          # [B,(r,i)]
        wk = W[r0:r1].transpose(0, 3, 1, 2).reshape(RI_LOC, CO)  # [(r,i),(c,o)]
        in_maps.append({
            "xt": np.ascontiguousarray(xk.T).astype(MM_NP),
            "xb": np.ascontiguousarray(xk).astype(MM_NP),
            "wg": np.ascontiguousarray(wk).astype(MM_NP),
            "biasf": biasf,
            "sel": sel,
            "selT": selT,
        })

    trace_cores = None
    if os.environ.get("K_TRACE_CORES"):
        trace_cores = [int(c) for c in os.environ["K_TRACE_CORES"].split(",")]

    global LAST_RESULT
    res = run_bass_kernel_spmd(
        _NC, in_maps, list(range(N_CORES)),
        trace=bool(os.environ.get("BASS_TRACE")),
        trace_cores=trace_cores,
    )
    LAST_RESULT = res
    if RDMA:
        # every core holds the full [128, NB*CO] result; b = kb*128 + p
        y0 = res.results[0]["y"].reshape(128, NB, CO)
        v = np.ascontiguousarray(y0.transpose(1, 0, 2)).reshape(B, CO)
    else:
        v = np.concatenate([res.results[k]["y"] for k in range(N_CORES)],
                           axis=0)
    return v.reshape(B, C, O)[..., None].astype(np.float32)


# revision 43
# speedup vs baseline: 1.4420x; 1.4420x over previous
"""Trainium2 Bass kernel for CapsNet dynamic routing (ClassCapsules).

Reference computation (B=256, R=1152, C=10, O=16, I=8, 3 routing iters):
    u_hat[b,r,c,o] = sum_i W[r,c,o,i] * x[b,r,i]
    b_ij = 0
    for it in 3:
        c_ij = softmax(b_ij, axis=1)                      # over c
        s = sum_r c_ij[r,c] * u_hat[b,r,c,o] + bias       # [B,C,O]
        v = squash(s)
        if it < 2:
            b_ij += mean_b sum_o u_hat[b,r,c,o] v[b,c,o]  # [R,C]
    return v[..., None]

u_hat ([B,R,C,O] = 189MB fp32) is never materialized.  Both routing
contractions are re-associated through the factorization
    s[b,co]    = x~[b,(ri)] @ (c∘W~)[(ri),(co)]
    agree[r,c] = sum_{i,o} W~[(ri),(co)] * G[(ri),(co)],
                 G = (1/B) x~^T v
with x~ = x viewed as [B, R*I] and W~ = W viewed as [R*I, C*O].

Distribution: R is sharded 8 ways (144 r's per core).  Per iteration the
partial s ([256,160], f16 on the wire) is summed across cores with one
AllReduce; the last iteration uses a ReduceScatter instead and each core
squashes + outputs its own 32-batch shard.  agree/b_ij/c_ij are fully
local to each core's r-shard.

Matmul inputs (x~, W~, c∘W~, v) are bf16: fp32 matmuls on TRN2 pay ~4x
in PE streaming and get no fast weight load.  PSUM accumulation stays
fp32 and the softmax/squash chain stays fp32, so only u_hat-level
rounding (~4e-3 relative) enters the routing.
"""

import os
import sys
import types

sys.path.insert(0, "/opt/trn_rl_repo")

# Shim antenv.axon_hooks (absent on this image) so BASS_TRACE=1 profiling
# works through run_bass_kernel_spmd's axon path.  Harmless when unused.
try:
    import antenv.axon_hooks  # noqa: F401
except ImportError:
    try:
        _hooks = types.ModuleType("antenv.axon_hooks")
        _hooks._hook = None
        _hooks.set_axon_ntff_profile_hook = lambda h: setattr(_hooks, "_hook", h)
        _hooks.get_axon_ntff_profile_hook = lambda: _hooks._hook
        sys.modules["antenv.axon_hooks"] = _hooks
        import antenv
        antenv.axon_hooks = _hooks
        from trn_agent_boot.trn_boot import _ntff_profile_via_ctypes
        _hooks.set_axon_ntff_profile_hook(
            _ntff_profile_via_ctypes("/opt/axon/libaxon_pjrt.so")
        )
    except Exception:
        pass

import numpy as np
import ml_dtypes

import concourse.bacc as bacc
import concourse.bass as bass
import concourse.tile as tile
from concourse import library_config, mybir
import concourse.bass_utils as _bass_utils
from concourse.bass_utils import run_bass_kernel_spmd

if os.environ.get("BASS_TRACE"):
    _bass_utils.upload_artifacts = lambda tmpdir: ""  # no bucket access here

LAST_RESULT = None

F32 = mybir.dt.float32
F16 = mybir.dt.float16
BF16 = mybir.dt.bfloat16
ALU = mybir.AluOpType
ACT = mybir.ActivationFunctionType

B, R, C, O, I = 256, 1152, 10, 16, 8
CO = C * O                      # 160
N_CORES = 8
R_LOC = R // N_CORES            # 144
RI_LOC = R_LOC * I              # 1152
NG = RI_LOC // 128              # 9 groups of 128 (r,i) rows
NB = B // 128                   # 2 batch partition chunks
B_SHARD = B // N_CORES          # 32 batches output per core
ITERS = 3
RPG = 128 // I                  # 16 r's per group

# With CC exchanges the stream-init barrier attaches to the first real
# AllReduce; a separate warm-up AR only adds serialized CC-stream time.
# (It is required for K_RDMA=1: PJRT staggers collective-free NEFF
# launches by milliseconds.)
WARM_AR = os.environ.get(
    "K_WARM_AR", "1" if os.environ.get("K_RDMA", "0") == "1" else "0"
) == "1"
CC_F16 = os.environ.get("K_CC_F16", "1") == "1"
CC_DT = F16 if CC_F16 else F32
MM_BF16 = os.environ.get("K_BF16", "1") == "1"
MM_DT = BF16 if MM_BF16 else F32
MM_NP = ml_dtypes.bfloat16 if MM_BF16 else np.float32
N_WARM_MM = int(os.environ.get("K_WARM_MM", "24"))
WARM_N = int(os.environ.get("K_WARM_N", "512"))
# Phase-B warm matmuls (keyed on the AllReduce result) measured as a large
# regression: they sit in-order ahead of the G matmuls and delay them.
N_WARM_B = int(os.environ.get("K_WARM_B", "0"))
RDMA = os.environ.get("K_RDMA", "0") == "1"
GPB = int(os.environ.get("K_GPB", "3"))      # psum_g bufs


def _squash(nc, eps_sb, t, n_part, nb, pool, out_dt=F32):
    """v = t * n2/((1+n2)*sqrt(n2+eps)); t: [n_part, nb, CO], reduce over o."""
    nc_ = nb * C
    tf = t.rearrange("p nb co -> p (nb co)")
    sq = pool.tile([n_part, nb * CO], F32, tag="sq")
    nc.vector.tensor_mul(sq, tf, tf)
    n2 = pool.tile([n_part, nc_], F32, tag="n2")
    nc.vector.reduce_sum(
        n2, sq.rearrange("p (nb c o) -> p nb c o", nb=nb, c=C),
        axis=mybir.AxisListType.X,
    )
    rt = pool.tile([n_part, nc_], F32, tag="rt")
    nc.scalar.activation(rt, n2, ACT.Sqrt, bias=eps_sb[:n_part])
    den = pool.tile([n_part, nc_], F32, tag="den")
    nc.vector.scalar_tensor_tensor(
        out=den, in0=n2, scalar=1.0, in1=rt, op0=ALU.add, op1=ALU.mult,
    )
    rec = pool.tile([n_part, nc_], F32, tag="rec")
    nc.vector.reciprocal(rec, den)
    fac = pool.tile([n_part, nc_], F32, tag="fac")
    nc.vector.tensor_mul(fac, n2, rec)
    v = pool.tile([n_part, nb, CO], out_dt, tag="v")
    fac_b = fac.rearrange(
        "p (nb c one) -> p nb c one", nb=nb, c=C
    ).broadcast_to([n_part, nb, C, O])
    nc.vector.tensor_tensor(
        out=v.rearrange("p nb (c o) -> p nb c o", c=C),
        in0=t.rearrange("p nb (c o) -> p nb c o", c=C),
        in1=fac_b,
        op=ALU.mult,
    )
    return v


def build():
    nc = bacc.Bacc("TRN2", target_bir_lowering=False, debug=False,
                   num_devices=N_CORES)

    xt_d = nc.dram_tensor("xt", [RI_LOC, B], MM_DT, kind="ExternalInput")
    xb_d = nc.dram_tensor("xb", [B, RI_LOC], MM_DT, kind="ExternalInput")
    wg_d = nc.dram_tensor("wg", [RI_LOC, CO], MM_DT, kind="ExternalInput")
    bias_d = nc.dram_tensor("biasf", [CO], F32, kind="ExternalInput")
    sel_d = nc.dram_tensor("sel", [128, RPG], F32, kind="ExternalInput")
    selT_d = nc.dram_tensor("selT", [RPG, 128], F32, kind="ExternalInput")
    if RDMA:
        # every core outputs the full squashed v; the host slices
        y_d = nc.dram_tensor("y", [128, NB * CO], F32, kind="ExternalOutput")
    else:
        y_d = nc.dram_tensor("y", [B_SHARD, CO], F32, kind="ExternalOutput")

    rg = [list(range(N_CORES))]

    with tile.TileContext(nc) as tc:
        with (
            tc.tile_pool(name="singles", bufs=1) as singles,
            tc.tile_pool(name="cw_pool", bufs=2) as cw_pool,
            tc.tile_pool(name="work", bufs=2) as work,
            tc.tile_pool(name="small", bufs=3) as small,
            tc.tile_pool(name="psum_s", bufs=1, space="PSUM") as psum_s,
            tc.tile_pool(name="psum_g", bufs=GPB, space="PSUM") as psum_g,
            tc.tile_pool(name="psum_misc", bufs=1, space="PSUM") as psum_misc,
            tc.tile_pool(name="dram", bufs=2, space="DRAM") as dram,
        ):
            rdma_waits = []
            if RDMA:
                nc.gpsimd.load_library(library_config.remote_dma)
                rsems = [nc.alloc_semaphore(f"rsem{e}") for e in range(ITERS)]
                lsem = nc.alloc_semaphore("lsem")
                # recv[e][:, d, :] is written by the peer whose (rid, tpb)
                # XOR-differs by d; slot 0 is the local partial (also the
                # broadcast source).  One buffer per exchange, never reused.
                recvs = [
                    singles.tile([128, N_CORES, NB * CO], CC_DT,
                                 name=f"recv{e}")
                    for e in range(ITERS)
                ]

            if WARM_AR:
                warm_sb = singles.tile([1, 8], F32)
                nc.vector.memset(warm_sb, 0.0)
                warm_in = dram.tile([8], F32)
                warm_out = dram.tile([8], F32)
                nc.gpsimd.dma_start(out=warm_in[:], in_=warm_sb[0, :])
                nc.gpsimd.collective_compute(
                    "AllReduce", ALU.add, replica_groups=rg,
                    ins=[warm_in[:]], outs=[warm_out[:]],
                )

            # ---- load inputs ----
            XT = singles.tile([128, NG, B], MM_DT)     # x~ [(ri),b] chunked
            nc.sync.dma_start(
                out=XT, in_=xt_d.ap().rearrange("(g p) b -> p g b", p=128)
            )
            XB = []                                    # x [b,(ri)] 2 p-chunks
            for kb in range(NB):
                t = singles.tile([128, RI_LOC], MM_DT, tag=f"xb{kb}",
                                 name=f"xb_sb{kb}")
                nc.sync.dma_start(out=t, in_=xb_d[kb * 128:(kb + 1) * 128, :])
                XB.append(t)
            WG = singles.tile([128, NG, CO], MM_DT)    # W~ [(ri),(co)] chunked
            nc.sync.dma_start(
                out=WG, in_=wg_d.ap().rearrange("(g p) n -> p g n", p=128)
            )
            biasb = singles.tile([128, CO], F32)
            nc.sync.dma_start(
                out=biasb,
                in_=bass.AP(tensor=bias_d, offset=0, ap=[[0, 128], [1, CO]]),
            )
            sel_sb = singles.tile([128, RPG], F32)
            nc.sync.dma_start(out=sel_sb, in_=sel_d[:, :])
            selT_sb = singles.tile([RPG, 128], F32)
            nc.sync.dma_start(out=selT_sb, in_=selT_d[:, :])
            if MM_BF16:
                sel_mm = singles.tile([128, RPG], MM_DT, name="sel_mm")
                nc.scalar.copy(sel_mm, sel_sb)
            else:
                sel_mm = sel_sb
            sel_cc = singles.tile([128, RPG], CC_DT, name="sel_cc")
            nc.scalar.copy(sel_cc, sel_sb)

            eps_sb = singles.tile([128, 1], F32)
            nc.vector.memset(eps_sb, 1e-8)

            esr = None   # [16, 90]: running exp(b_ij)

            for it in range(ITERS):
                # ---- CW = c∘W~ (it>0); it=0 uses uniform c=0.1 folded later
                if it == 0:
                    CW = WG
                else:
                    # cnorm[16, 90] = esr * (1/sum_c esr), broadcast to the
                    # 128 (r,i) partition rows via the selT matmul.
                    cp_ps = psum_misc.tile([128, NG * C], F32, tag="cp",
                                           name=f"cp_ps_{it}")
                    nc.tensor.matmul(cp_ps, selT_sb, esr, start=True,
                                     stop=True)
                    cp_sb = small.tile([128, NG * C], MM_DT, tag="cpart",
                                       name=f"cp_sb_{it}")
                    nc.scalar.copy(cp_sb, cp_ps)
                    CW = cw_pool.tile([128, NG, CO], MM_DT, tag="cw",
                                      name=f"cw_{it}")
                    c_b = cp_sb.rearrange(
                        "p (g c one) -> p g c one", g=NG, one=1
                    ).broadcast_to([128, NG, C, O])
                    nc.vector.tensor_tensor(
                        out=CW.rearrange("p g (c o) -> p g c o", c=C),
                        in0=WG.rearrange("p g (c o) -> p g c o", c=C),
                        in1=c_b, op=ALU.mult,
                    )

                # ---- s partial: [256,160] = x~^T @ CW, K = (ri) local ----
                s_ps = [psum_s.tile([128, CO], F32, tag=f"s{kb}",
                                    name=f"s_ps{kb}_{it}")
                        for kb in range(NB)]
                for kb in range(NB):
                    for g in range(NG):
                        nc.tensor.matmul(
                            s_ps[kb],
                            XT[:, g, kb * 128:(kb + 1) * 128],
                            CW[:, g, :],
                            start=(g == 0),
                            stop=(g == NG - 1),
                        )

                if RDMA:
                    recv = recvs[it]
                    rsem = rsems[it]
                    cp_insts = []
                    for kb in range(NB):
                        cp_insts.append(nc.scalar.copy(
                            recv[:, 0, kb * CO:(kb + 1) * CO], s_ps[kb]
                        ))
                    for dd in range(1, N_CORES):
                        rd: list = [None] * N_CORES
                        rd[dd] = (0, dd)
                        nc.gpsimd.remote_dma_broadcast(
                            out_ap=recv[:, dd, :],
                            in_ap=recv[:, 0, :],
                            remote_sem=rsem,
                            local_sem=lsem,
                            rdests=rd,
                        )
                    nc.gpsimd.trigger_dma(count=None)
                    if N_WARM_MM and it < ITERS - 1:
                        warm_ps = psum_misc.tile([RPG, 512], F32,
                                                 tag="warmps",
                                                 name=f"warm_ps_{it}")
                        warm_rhs = XT.rearrange("p g b -> p (g b)")[:, :WARM_N]
                        for wi in range(N_WARM_MM):
                            nc.tensor.matmul(
                                warm_ps[:, :WARM_N], sel_mm, warm_rhs,
                                start=(wi == 0), stop=True,
                                skip_group_check=True,
                            )
                    # tree-reduce the 8 slots; the first add must wait for
                    # the 7 remote arrivals (2 sem increments each).  The
                    # wait is injected post-compile (see build() tail): the
                    # single-core scheduling sim can never see remote sem
                    # updates and would report a deadlock.
                    # The arrival gate (wait rsem>=14) is spliced in front of
                    # this reduce post-compile: the single-core scheduling
                    # sim cannot see remote sem updates and would deadlock,
                    # and compute instructions carry at most one wait slot.
                    red = work.tile([128, NB, CO], F32, tag="red",
                                    name=f"red_{it}")
                    red_i = nc.vector.reduce_sum(
                        red.rearrange("p nb co -> p (nb co)"),
                        recv.rearrange("p s x -> p x s"),
                        axis=mybir.AxisListType.X,
                    )
                    rdma_waits.append((red_i.ins.name, rsem))
                    t = work.tile([128, NB, CO], F32, tag="t",
                                  name=f"t_{it}")
                    bias_b = biasb.rearrange(
                        "p (one co) -> p one co", one=1
                    ).broadcast_to([128, NB, CO])
                    nc.vector.scalar_tensor_tensor(
                        out=t, in0=red,
                        scalar=(0.1 if it == 0 else 1.0),
                        in1=bias_b, op0=ALU.mult, op1=ALU.add,
                    )
                    if it < ITERS - 1:
                        v_sb = _squash(nc, eps_sb, t, 128, NB, work,
                                       out_dt=MM_DT)
                    else:
                        v_out = _squash(nc, eps_sb, t, 128, NB, work)
                        nc.sync.dma_start(
                            out=y_d[:, :],
                            in_=v_out.rearrange("p nb co -> p (nb co)"),
                        )
                else:
                    cc_in = dram.tile([NB, 128, CO], CC_DT, tag="cc_in",
                                      name=f"cc_in_{it}")
                    for kb in range(NB):
                        s_stage = work.tile([128, CO], CC_DT,
                                            tag=f"sstage{kb}",
                                            name=f"s_stage{kb}_{it}")
                        nc.scalar.copy(s_stage, s_ps[kb])
                        nc.sync.dma_start(out=cc_in[kb, :, :], in_=s_stage)
                        if kb == NB - 1:
                            dsq = small.tile([1, 1], F32, tag="dsq",
                                             name=f"dsq_{it}")
                            nc.scalar.activation(dsq, s_stage[:1, :1],
                                                 ACT.Sqrt,
                                                 bias=eps_sb[:1], scale=0.0)

                if not RDMA and it < ITERS - 1:
                    # ---- AllReduce s; every core squashes the full batch
                    cc_out = dram.tile([NB, 128, CO], CC_DT, tag="cc_out",
                                       name=f"cc_out_{it}")
                    nc.gpsimd.collective_compute(
                        "AllReduce", ALU.add, replica_groups=rg,
                        ins=[cc_in.opt()], outs=[cc_out.opt()],
                    )
                    s_sb = work.tile([128, NB, CO], CC_DT, tag="ssb",
                                     name=f"s_sb_{it}")
                    for kb in range(NB):
                        nc.sync.dma_start(
                            out=s_sb[:, kb, :], in_=cc_out[kb, :, :]
                        )
                    if N_WARM_MM:
                        # Phase A: dependency-free burst keeps the PE's HAM
                        # window busy through the AllReduce wait.
                        warm_ps = psum_misc.tile([RPG, 512], F32,
                                                 tag="warmps",
                                                 name=f"warm_ps_{it}")
                        warm_rhs = XT.rearrange(
                            "p g b -> p (g b)")[:, :WARM_N]
                        for wi in range(N_WARM_MM):
                            nc.tensor.matmul(
                                warm_ps[:, :WARM_N], sel_mm, warm_rhs,
                                start=(wi == 0), stop=True,
                                skip_group_check=True,
                            )
                        # Phase B: keyed on the AllReduce result, so it
                        # spans the squash chain and hands the G matmuls a
                        # warm (2.4 GHz) PE.  Output is discarded.
                        warm_b = s_sb.rearrange("p nb co -> p (nb co)")
                        for wi in range(N_WARM_B):
                            nc.tensor.matmul(
                                warm_ps[:, :NB * CO], sel_cc, warm_b,
                                start=(wi == 0), stop=True,
                                skip_group_check=True,
                            )
                    t = work.tile([128, NB, CO], F32, tag="t",
                                  name=f"t_{it}")
                    bias_b = biasb.rearrange(
                        "p (one co) -> p one co", one=1
                    ).broadcast_to([128, NB, CO])
                    nc.vector.scalar_tensor_tensor(
                        out=t, in0=s_sb,
                        scalar=(0.1 if it == 0 else 1.0),
                        in1=bias_b, op0=ALU.mult, op1=ALU.add,
                    )
                    v_sb = _squash(nc, eps_sb, t, 128, NB, work,
                                   out_dt=MM_DT)

                if it < ITERS - 1:
                    # ---- G = (1/B) x~^T v ; agree = sum_io W∘G ----
                    # PSUM banks are drained by the scalar engine (idle
                    # here), freeing the DVE for the squash and keeping the
                    # PE from stalling on bank recycling; the W∘G multiply
                    # then runs as one big stt.
                    Q_all = small.tile([128, NG * C], MM_DT, tag="qall",
                                       name=f"qall_{it}")
                    g_all = work.tile([128, NG, CO], F32, tag="gall",
                                      name=f"gall_{it}")
                    p9 = work.tile([128, NG, CO], MM_DT, tag="p9",
                                   name=f"p9_{it}")
                    for g in range(NG):
                        g_ps = psum_g.tile([128, CO], F32, tag="gps",
                                           name=f"g_ps_{it}_{g}")
                        for kb in range(NB):
                            nc.tensor.matmul(
                                g_ps,
                                XB[kb][:, g * 128:(g + 1) * 128],
                                v_sb[:, kb, :],
                                start=(kb == 0),
                                stop=(kb == NB - 1),
                            )
                        nc.scalar.copy(g_all[:, g, :], g_ps)
                    nc.vector.scalar_tensor_tensor(
                        out=p9, in0=g_all, scalar=1.0 / B,
                        in1=WG, op0=ALU.mult, op1=ALU.mult,
                    )
                    # bf16 accumulation over the 16 o's: ~0.4% on agree,
                    # well inside the 2e-2 budget, and 2x DVE throughput.
                    with nc.allow_low_precision(reason="agree tolerates bf16"):
                        for lo, hi in ((0, 4), (4, 8), (8, 9)):
                            nc.vector.reduce_sum(
                                Q_all[:, lo * C:hi * C],
                                p9[:, lo:hi, :].rearrange(
                                    "p g (c o) -> p (g c) o", c=C),
                                axis=mybir.AxisListType.X,
                            )
                    agree_ps = psum_misc.tile([RPG, NG * C], F32, tag="agree",
                                              name=f"agree_{it}")
                    nc.tensor.matmul(agree_ps, sel_mm, Q_all,
                                     start=True, stop=True)

                    # ---- softmax logits, chained multiplicatively:
                    # softmax(b_prev + agree) ∝ c_prev_normalized * exp(agree)
                    # (per-row scale from early normalization cancels) ----
                    eexp = small.tile([RPG, NG * C], F32, tag="eexp",
                                      name=f"eexp_{it}")
                    nc.scalar.activation(eexp, agree_ps, ACT.Exp)
                    if it == 0:
                        base = eexp
                    else:
                        base = small.tile([RPG, NG * C], F32, tag="esrr",
                                          name=f"base_{it}")
                        nc.vector.tensor_mul(base, esr, eexp)
                    den = small.tile([RPG, NG], F32, tag="sden",
                                     name=f"den_{it}")
                    nc.vector.reduce_sum(
                        den,
                        base.rearrange("p (g c) -> p g c", g=NG),
                        axis=mybir.AxisListType.X,
                    )
                    rec_r = small.tile([RPG, NG], F32, tag="srec",
                                       name=f"rec_{it}")
                    nc.vector.reciprocal(rec_r, den)
                    esr_n = small.tile([RPG, NG * C], F32, tag="esr",
                                       name=f"esr_{it}")
                    rec_b = rec_r.rearrange(
                        "p (g one) -> p g one", one=1
                    ).broadcast_to([RPG, NG, C])
                    nc.vector.tensor_tensor(
                        out=esr_n.rearrange("p (g c) -> p g c", g=NG),
                        in0=base.rearrange("p (g c) -> p g c", g=NG),
                        in1=rec_b, op=ALU.mult,
                    )
                    esr = esr_n       # normalized c_ij rows, [16, 90]
                elif not RDMA:
                    # ---- final iter: ReduceScatter; squash own b-shard ----
                    rs_out = dram.tile([B_SHARD * CO], CC_DT, tag="rs_out")
                    nc.gpsimd.collective_compute(
                        "ReduceScatter", ALU.add, replica_groups=rg,
                        ins=[cc_in.opt()], outs=[rs_out[:]],
                    )
                    s_sb = work.tile([B_SHARD, 1, CO], CC_DT, tag="fs")
                    nc.sync.dma_start(
                        out=s_sb,
                        in_=rs_out.rearrange("(p one n) -> p one n",
                                             n=CO, one=1),
                    )
                    t = work.tile([B_SHARD, 1, CO], F32, tag="ft")
                    bias_b1 = biasb[:B_SHARD, :].rearrange(
                        "p (one co) -> p one co", one=1
                    )
                    nc.vector.scalar_tensor_tensor(
                        out=t, in0=s_sb, scalar=1.0,
                        in1=bias_b1, op0=ALU.mult, op1=ALU.add,
                    )
                    v = _squash(nc, eps_sb, t, B_SHARD, 1, work)
                    nc.sync.dma_start(
                        out=y_d[:, :], in_=v.rearrange("p one co -> p (one co)")
                    )

    nc.compile()
    # Splice the remote-arrival gates in front of each slot reduce, now
    # that scheduling is done.  A standalone EventSemaphore on the DVE
    # queue stalls it (in-order) until the 7 remote transfers landed.
    f = nc.m.functions[0]
    for r4_name, sem in rdma_waits:
        w = nc.vector.wait_ge(sem, 14)
        w_ins = None
        for b in f.blocks:
            if b.instructions and b.instructions[-1] is w.ins:
                w_ins = b.instructions.pop()
                break
        assert w_ins is not None, "gate wait not found at a block tail"
        placed = False
        for b in f.blocks:
            for idx, i in enumerate(b.instructions):
                if i.name == r4_name:
                    b.instructions.insert(idx, w_ins)
                    placed = True
                    break
            if placed:
                break
        assert placed, f"reduce {r4_name} not found for gate splice"
    return nc


_NC = None


def kernel(x: np.ndarray, W: np.ndarray, bias: np.ndarray) -> np.ndarray:
    global _NC
    if _NC is None:
        _NC = build()

    x = np.ascontiguousarray(x, dtype=np.float32)
    W = np.ascontiguousarray(W, dtype=np.float32)
    bias = np.ascontiguousarray(bias, dtype=np.float32)

    biasf = bias.reshape(CO)
    sel = np.zeros((128, RPG), dtype=np.float32)
    sel[np.arange(128), np.arange(128) // I] = 1.0
    selT = np.ascontiguousarray(sel.T)

    in_maps = []
    for k in range(N_CORES):
        r0, r1 = k * R_LOC, (k + 1) * R_LOC
        xk = x[:, r0:r1, :].reshape(B, RI_LOC)          # [B,(r,i)]
        wk = W[r0:r1].transpose(0, 3, 1, 2).reshape(RI_LOC, CO)  # [(r,i),(c,o)]
        in_maps.append({
            "xt": np.ascontiguousarray(xk.T).astype(MM_NP),
            "xb": np.ascontiguousarray(xk).astype(MM_NP),
            "wg": np.ascontiguousarray(wk).astype(MM_NP),
            "biasf": biasf,
            "sel": sel,
            "selT": selT,
        })

    trace_cores = None
    if os.environ.get("K_TRACE_CORES"):
        trace_cores = [int(c) for c in os.environ["K_TRACE_CORES"].split(",")]

    global LAST_RESULT
    res = run_bass_kernel_spmd(
        _NC, in_maps, list(range(N_CORES)),
        trace=bool(os.environ.get("BASS_TRACE")),
        trace_cores=trace_cores,
    )
    LAST_RESULT = res
    if RDMA:
        # every core holds the full [128, NB*CO] result; b = kb*128 + p
        y0 = res.results[0]["y"].reshape(128, NB, CO)
        v = np.ascontiguousarray(y0.transpose(1, 0, 2)).reshape(B, CO)
    else:
        v = np.concatenate([res.results[k]["y"] for k in range(N_CORES)],
                           axis=0)
    return v.reshape(B, C, O)[..., None].astype(np.float32)


# revision 46
# speedup vs baseline: 1.4593x; 1.0120x over previous
"""Trainium2 Bass kernel for CapsNet dynamic routing (ClassCapsules).

Reference computation (B=256, R=1152, C=10, O=16, I=8, 3 routing iters):
    u_hat[b,r,c,o] = sum_i W[r,c,o,i] * x[b,r,i]
    b_ij = 0
    for it in 3:
        c_ij = softmax(b_ij, axis=1)                      # over c
        s = sum_r c_ij[r,c] * u_hat[b,r,c,o] + bias       # [B,C,O]
        v = squash(s)
        if it < 2:
            b_ij += mean_b sum_o u_hat[b,r,c,o] v[b,c,o]  # [R,C]
    return v[..., None]

u_hat ([B,R,C,O] = 189MB fp32) is never materialized.  Both routing
contractions are re-associated through the factorization
    s[b,co]    = x~[b,(ri)] @ (c∘W~)[(ri),(co)]
    agree[r,c] = sum_{i,o} W~[(ri),(co)] * G[(ri),(co)],
                 G = (1/B) x~^T v
with x~ = x viewed as [B, R*I] and W~ = W viewed as [R*I, C*O].

Distribution: R is sharded 8 ways (144 r's per core).  Per iteration the
partial s ([256,160], f16 on the wire) is summed across cores with one
AllReduce; the last iteration uses a ReduceScatter instead and each core
squashes + outputs its own 32-batch shard.  agree/b_ij/c_ij are fully
local to each core's r-shard.

Matmul inputs (x~, W~, c∘W~, v) are bf16: fp32 matmuls on TRN2 pay ~4x
in PE streaming and get no fast weight load.  PSUM accumulation stays
fp32 and the softmax/squash chain stays fp32, so only u_hat-level
rounding (~4e-3 relative) enters the routing.
"""

import os
import sys
import types

sys.path.insert(0, "/opt/trn_rl_repo")

# Shim antenv.axon_hooks (absent on this image) so BASS_TRACE=1 profiling
# works through run_bass_kernel_spmd's axon path.  Harmless when unused.
try:
    import antenv.axon_hooks  # noqa: F401
except ImportError:
    try:
        _hooks = types.ModuleType("antenv.axon_hooks")
        _hooks._hook = None
        _hooks.set_axon_ntff_profile_hook = lambda h: setattr(_hooks, "_hook", h)
        _hooks.get_axon_ntff_profile_hook = lambda: _hooks._hook
        sys.modules["antenv.axon_hooks"] = _hooks
        import antenv
        antenv.axon_hooks = _hooks
        from trn_agent_boot.trn_boot import _ntff_profile_via_ctypes
        _hooks.set_axon_ntff_profile_hook(
            _ntff_profile_via_ctypes("/opt/axon/libaxon_pjrt.so")
        )
    except Exception:
        pass

import numpy as np
import ml_dtypes

import concourse.bacc as bacc
import concourse.bass as bass
import concourse.tile as tile
from concourse import library_config, mybir
import concourse.bass_utils as _bass_utils
from concourse.bass_utils import run_bass_kernel_spmd

if os.environ.get("BASS_TRACE"):
    _bass_utils.upload_artifacts = lambda tmpdir: ""  # no bucket access here

LAST_RESULT = None

F32 = mybir.dt.float32
F16 = mybir.dt.float16
BF16 = mybir.dt.bfloat16
ALU = mybir.AluOpType
ACT = mybir.ActivationFunctionType

B, R, C, O, I = 256, 1152, 10, 16, 8
CO = C * O                      # 160
N_CORES = 8
R_LOC = R // N_CORES            # 144
RI_LOC = R_LOC * I              # 1152
NG = RI_LOC // 128              # 9 groups of 128 (r,i) rows
NB = B // 128                   # 2 batch partition chunks
B_SHARD = B // N_CORES          # 32 batches output per core
ITERS = 3
RPG = 128 // I                  # 16 r's per group

# With CC exchanges the stream-init barrier attaches to the first real
# AllReduce; a separate warm-up AR only adds serialized CC-stream time.
# (It is required for K_RDMA=1: PJRT staggers collective-free NEFF
# launches by milliseconds.)
WARM_AR = os.environ.get(
    "K_WARM_AR", "1" if os.environ.get("K_RDMA", "0") == "1" else "0"
) == "1"
CC_F16 = os.environ.get("K_CC_F16", "1") == "1"
CC_DT = F16 if CC_F16 else F32
MM_BF16 = os.environ.get("K_BF16", "1") == "1"
MM_DT = BF16 if MM_BF16 else F32
MM_NP = ml_dtypes.bfloat16 if MM_BF16 else np.float32
N_WARM_MM = int(os.environ.get("K_WARM_MM", "26"))
WARM_N = int(os.environ.get("K_WARM_N", "512"))
RDMA = os.environ.get("K_RDMA", "0") == "1"
GPB = int(os.environ.get("K_GPB", "3"))      # psum_g bufs


def _squash(nc, eps_sb, t, n_part, nb, pool, out_dt=F32):
    """v = t * n2/((1+n2)*sqrt(n2+eps)); t: [n_part, nb, CO], reduce over o."""
    nc_ = nb * C
    tf = t.rearrange("p nb co -> p (nb co)")
    sq = pool.tile([n_part, nb * CO], F32, tag="sq")
    nc.vector.tensor_mul(sq, tf, tf)
    n2 = pool.tile([n_part, nc_], F32, tag="n2")
    nc.vector.reduce_sum(
        n2, sq.rearrange("p (nb c o) -> p nb c o", nb=nb, c=C),
        axis=mybir.AxisListType.X,
    )
    rt = pool.tile([n_part, nc_], F32, tag="rt")
    nc.scalar.activation(rt, n2, ACT.Sqrt, bias=eps_sb[:n_part])
    den = pool.tile([n_part, nc_], F32, tag="den")
    nc.vector.scalar_tensor_tensor(
        out=den, in0=n2, scalar=1.0, in1=rt, op0=ALU.add, op1=ALU.mult,
    )
    rec = pool.tile([n_part, nc_], F32, tag="rec")
    nc.vector.reciprocal(rec, den)
    fac = pool.tile([n_part, nc_], F32, tag="fac")
    nc.vector.tensor_mul(fac, n2, rec)
    v = pool.tile([n_part, nb, CO], out_dt, tag="v")
    fac_b = fac.rearrange(
        "p (nb c one) -> p nb c one", nb=nb, c=C
    ).broadcast_to([n_part, nb, C, O])
    nc.vector.tensor_tensor(
        out=v.rearrange("p nb (c o) -> p nb c o", c=C),
        in0=t.rearrange("p nb (c o) -> p nb c o", c=C),
        in1=fac_b,
        op=ALU.mult,
    )
    return v


def build():
    nc = bacc.Bacc("TRN2", target_bir_lowering=False, debug=False,
                   num_devices=N_CORES)

    xt_d = nc.dram_tensor("xt", [RI_LOC, B], MM_DT, kind="ExternalInput")
    xb_d = nc.dram_tensor("xb", [B, RI_LOC], MM_DT, kind="ExternalInput")
    wg_d = nc.dram_tensor("wg", [RI_LOC, CO], MM_DT, kind="ExternalInput")
    bias_d = nc.dram_tensor("biasf", [CO], F32, kind="ExternalInput")
    sel_d = nc.dram_tensor("sel", [128, RPG], F32, kind="ExternalInput")
    selT_d = nc.dram_tensor("selT", [RPG, 128], F32, kind="ExternalInput")
    if RDMA:
        # every core outputs the full squashed v; the host slices
        y_d = nc.dram_tensor("y", [128, NB * CO], F32, kind="ExternalOutput")
    else:
        y_d = nc.dram_tensor("y", [B_SHARD, CO], F32, kind="ExternalOutput")

    rg = [list(range(N_CORES))]

    with tile.TileContext(nc) as tc:
        with (
            tc.tile_pool(name="singles", bufs=1) as singles,
            tc.tile_pool(name="cw_pool", bufs=2) as cw_pool,
            tc.tile_pool(name="work", bufs=2) as work,
            tc.tile_pool(name="small", bufs=3) as small,
            tc.tile_pool(name="psum_s", bufs=1, space="PSUM") as psum_s,
            tc.tile_pool(name="psum_g", bufs=GPB, space="PSUM") as psum_g,
            tc.tile_pool(name="psum_misc", bufs=1, space="PSUM") as psum_misc,
            tc.tile_pool(name="dram", bufs=2, space="DRAM") as dram,
        ):
            rdma_waits = []
            if RDMA:
                nc.gpsimd.load_library(library_config.remote_dma)
                rsems = [nc.alloc_semaphore(f"rsem{e}") for e in range(ITERS)]
                lsem = nc.alloc_semaphore("lsem")
                # recv[e][:, d, :] is written by the peer whose (rid, tpb)
                # XOR-differs by d; slot 0 is the local partial (also the
                # broadcast source).  One buffer per exchange, never reused.
                recvs = [
                    singles.tile([128, N_CORES, NB * CO], CC_DT,
                                 name=f"recv{e}")
                    for e in range(ITERS)
                ]

            if WARM_AR:
                warm_sb = singles.tile([1, 8], F32)
                nc.vector.memset(warm_sb, 0.0)
                warm_in = dram.tile([8], F32)
                warm_out = dram.tile([8], F32)
                nc.gpsimd.dma_start(out=warm_in[:], in_=warm_sb[0, :])
                nc.gpsimd.collective_compute(
                    "AllReduce", ALU.add, replica_groups=rg,
                    ins=[warm_in[:]], outs=[warm_out[:]],
                )

            # ---- load inputs ----
            # XT and WG feed the iteration-0 s matmuls, which gate the AR0
            # trigger (and the CC barrier tracks the trigger) — load them
            # first.  XB is only needed by the G phase after AR0; it goes
            # last and on the gpsimd DMA queue so it never delays s0.
            XT = singles.tile([128, NG, B], MM_DT)     # x~ [(ri),b] chunked
            nc.sync.dma_start(
                out=XT, in_=xt_d.ap().rearrange("(g p) b -> p g b", p=128)
            )
            WG = singles.tile([128, NG, CO], MM_DT)    # W~ [(ri),(co)] chunked
            nc.sync.dma_start(
                out=WG, in_=wg_d.ap().rearrange("(g p) n -> p g n", p=128)
            )
            XB = []                                    # x [b,(ri)] 2 p-chunks
            for kb in range(NB):
                t = singles.tile([128, RI_LOC], MM_DT, tag=f"xb{kb}",
                                 name=f"xb_sb{kb}")
                nc.gpsimd.dma_start(out=t,
                                    in_=xb_d[kb * 128:(kb + 1) * 128, :])
                XB.append(t)
            biasb = singles.tile([128, CO], F32)
            nc.sync.dma_start(
                out=biasb,
                in_=bass.AP(tensor=bias_d, offset=0, ap=[[0, 128], [1, CO]]),
            )
            sel_sb = singles.tile([128, RPG], F32)
            nc.sync.dma_start(out=sel_sb, in_=sel_d[:, :])
            selT_sb = singles.tile([RPG, 128], F32)
            nc.sync.dma_start(out=selT_sb, in_=selT_d[:, :])
            if MM_BF16:
                sel_mm = singles.tile([128, RPG], MM_DT, name="sel_mm")
                nc.scalar.copy(sel_mm, sel_sb)
            else:
                sel_mm = sel_sb

            eps_sb = singles.tile([128, 1], F32)
            nc.vector.memset(eps_sb, 1e-8)

            esr = None   # [16, 90]: running exp(b_ij)

            for it in range(ITERS):
                # ---- CW = c∘W~ (it>0); it=0 uses uniform c=0.1 folded later
                if it == 0:
                    CW = WG
                else:
                    # cnorm[16, 90] = esr * (1/sum_c esr), broadcast to the
                    # 128 (r,i) partition rows via the selT matmul.
                    cp_ps = psum_misc.tile([128, NG * C], F32, tag="cp",
                                           name=f"cp_ps_{it}")
                    nc.tensor.matmul(cp_ps, selT_sb, esr, start=True,
                                     stop=True)
                    cp_sb = small.tile([128, NG * C], F32, tag="cpart",
                                       name=f"cp_sb_{it}")
                    nc.scalar.copy(cp_sb, cp_ps)
                    CW = cw_pool.tile([128, NG, CO], MM_DT, tag="cw",
                                      name=f"cw_{it}")
                    # split into 3-group chunks so the s matmuls can start
                    # streaming group 0 while the rest is still multiplying
                    c_b = cp_sb.rearrange(
                        "p (g c one) -> p g c one", g=NG, one=1
                    ).broadcast_to([128, NG, C, O])
                    cw4 = CW.rearrange("p g (c o) -> p g c o", c=C)
                    wg4 = WG.rearrange("p g (c o) -> p g c o", c=C)
                    for lo in range(0, NG, 3):
                        hi = min(lo + 3, NG)
                        nc.vector.tensor_tensor(
                            out=cw4[:, lo:hi], in0=wg4[:, lo:hi],
                            in1=c_b[:, lo:hi], op=ALU.mult,
                        )

                # ---- s partial: [256,160] = x~^T @ CW, K = (ri) local ----
                s_ps = [psum_s.tile([128, CO], F32, tag=f"s{kb}",
                                    name=f"s_ps{kb}_{it}")
                        for kb in range(NB)]
                for kb in range(NB):
                    for g in range(NG):
                        nc.tensor.matmul(
                            s_ps[kb],
                            XT[:, g, kb * 128:(kb + 1) * 128],
                            CW[:, g, :],
                            start=(g == 0),
                            stop=(g == NG - 1),
                        )

                if RDMA:
                    recv = recvs[it]
                    rsem = rsems[it]
                    cp_insts = []
                    for kb in range(NB):
                        cp_insts.append(nc.scalar.copy(
                            recv[:, 0, kb * CO:(kb + 1) * CO], s_ps[kb]
                        ))
                    for dd in range(1, N_CORES):
                        rd: list = [None] * N_CORES
                        rd[dd] = (0, dd)
                        nc.gpsimd.remote_dma_broadcast(
                            out_ap=recv[:, dd, :],
                            in_ap=recv[:, 0, :],
                            remote_sem=rsem,
                            local_sem=lsem,
                            rdests=rd,
                        )
                    nc.gpsimd.trigger_dma(count=None)
                    if N_WARM_MM and it < ITERS - 1:
                        warm_ps = psum_misc.tile([RPG, 512], F32,
                                                 tag="warmps",
                                                 name=f"warm_ps_{it}")
                        warm_rhs = XT.rearrange("p g b -> p (g b)")[:, :WARM_N]
                        for wi in range(N_WARM_MM):
                            nc.tensor.matmul(
                                warm_ps[:, :WARM_N], sel_mm, warm_rhs,
                                start=(wi == 0), stop=True,
                                skip_group_check=True,
                            )
                    # tree-reduce the 8 slots; the first add must wait for
                    # the 7 remote arrivals (2 sem increments each).  The
                    # wait is injected post-compile (see build() tail): the
                    # single-core scheduling sim can never see remote sem
                    # updates and would report a deadlock.
                    # The arrival gate (wait rsem>=14) is spliced in front of
                    # this reduce post-compile: the single-core scheduling
                    # sim cannot see remote sem updates and would deadlock,
                    # and compute instructions carry at most one wait slot.
                    red = work.tile([128, NB, CO], F32, tag="red",
                                    name=f"red_{it}")
                    red_i = nc.vector.reduce_sum(
                        red.rearrange("p nb co -> p (nb co)"),
                        recv.rearrange("p s x -> p x s"),
                        axis=mybir.AxisListType.X,
                    )
                    rdma_waits.append((red_i.ins.name, rsem))
                    t = work.tile([128, NB, CO], F32, tag="t",
                                  name=f"t_{it}")
                    bias_b = biasb.rearrange(
                        "p (one co) -> p one co", one=1
                    ).broadcast_to([128, NB, CO])
                    nc.vector.scalar_tensor_tensor(
                        out=t, in0=red,
                        scalar=(0.1 if it == 0 else 1.0),
                        in1=bias_b, op0=ALU.mult, op1=ALU.add,
                    )
                    if it < ITERS - 1:
                        v_sb = _squash(nc, eps_sb, t, 128, NB, work,
                                       out_dt=MM_DT)
                    else:
                        v_out = _squash(nc, eps_sb, t, 128, NB, work)
                        nc.sync.dma_start(
                            out=y_d[:, :],
                            in_=v_out.rearrange("p nb co -> p (nb co)"),
                        )
                else:
                    cc_in = dram.tile([NB, 128, CO], CC_DT, tag="cc_in",
                                      name=f"cc_in_{it}")
                    for kb in range(NB):
                        s_stage = work.tile([128, CO], CC_DT,
                                            tag=f"sstage{kb}",
                                            name=f"s_stage{kb}_{it}")
                        nc.scalar.copy(s_stage, s_ps[kb])
                        nc.sync.dma_start(out=cc_in[kb, :, :], in_=s_stage)
                        if kb == NB - 1:
                            dsq = small.tile([1, 1], F32, tag="dsq",
                                             name=f"dsq_{it}")
                            nc.scalar.activation(dsq, s_stage[:1, :1],
                                                 ACT.Sqrt,
                                                 bias=eps_sb[:1], scale=0.0)

                if not RDMA and it < ITERS - 1:
                    # ---- AllReduce s; every core squashes the full batch
                    cc_out = dram.tile([NB, 128, CO], CC_DT, tag="cc_out",
                                       name=f"cc_out_{it}")
                    nc.gpsimd.collective_compute(
                        "AllReduce", ALU.add, replica_groups=rg,
                        ins=[cc_in.opt()], outs=[cc_out.opt()],
                    )
                    s_sb = work.tile([128, NB, CO], CC_DT, tag="ssb",
                                     name=f"s_sb_{it}")
                    for kb in range(NB):
                        nc.sync.dma_start(
                            out=s_sb[:, kb, :], in_=cc_out[kb, :, :]
                        )
                    if N_WARM_MM:
                        warm_ps = psum_misc.tile([RPG, 512], F32,
                                                 tag="warmps",
                                                 name=f"warm_ps_{it}")
                        warm_rhs = XT.rearrange(
                            "p g b -> p (g b)")[:, :WARM_N]
                        for wi in range(N_WARM_MM):
                            nc.tensor.matmul(
                                warm_ps[:, :WARM_N], sel_mm, warm_rhs,
                                start=(wi == 0), stop=True,
                                skip_group_check=True,
                            )
                    t = work.tile([128, NB, CO], F32, tag="t",
                                  name=f"t_{it}")
                    bias_b = biasb.rearrange(
                        "p (one co) -> p one co", one=1
                    ).broadcast_to([128, NB, CO])
                    nc.vector.scalar_tensor_tensor(
                        out=t, in0=s_sb,
                        scalar=(0.1 if it == 0 else 1.0),
                        in1=bias_b, op0=ALU.mult, op1=ALU.add,
                    )
                    v_sb = _squash(nc, eps_sb, t, 128, NB, work,
                                   out_dt=MM_DT)

                if it < ITERS - 1:
                    # ---- G = (1/B) x~^T v ; agree = sum_io W∘G ----
                    # PSUM banks are drained by the scalar engine (idle
                    # here), freeing the DVE for the squash and keeping the
                    # PE from stalling on bank recycling; the W∘G multiply
                    # then runs as one big stt.
                    Q_all = small.tile([128, NG * C], F32, tag="qall",
                                       name=f"qall_{it}")
                    g_all = work.tile([128, NG, CO], F32, tag="gall",
                                      name=f"gall_{it}")
                    p9 = work.tile([128, NG, CO], F32, tag="p9",
                                   name=f"p9_{it}")
                    for g in range(NG):
                        g_ps = psum_g.tile([128, CO], F32, tag="gps",
                                           name=f"g_ps_{it}_{g}")
                        for kb in range(NB):
                            nc.tensor.matmul(
                                g_ps,
                                XB[kb][:, g * 128:(g + 1) * 128],
                                v_sb[:, kb, :],
                                start=(kb == 0),
                                stop=(kb == NB - 1),
                            )
                        nc.scalar.copy(g_all[:, g, :], g_ps)
                    nc.vector.scalar_tensor_tensor(
                        out=p9, in0=g_all, scalar=1.0 / B,
                        in1=WG, op0=ALU.mult, op1=ALU.mult,
                    )
                    for lo, hi in ((0, 4), (4, 8), (8, 9)):
                        nc.vector.reduce_sum(
                            Q_all[:, lo * C:hi * C],
                            p9[:, lo:hi, :].rearrange(
                                "p g (c o) -> p (g c) o", c=C),
                            axis=mybir.AxisListType.X,
                        )
                    agree_ps = psum_misc.tile([RPG, NG * C], F32, tag="agree",
                                              name=f"agree_{it}")
                    nc.tensor.matmul(agree_ps, sel_sb, Q_all,
                                     start=True, stop=True)

                    # ---- softmax logits, chained multiplicatively:
                    # softmax(b_prev + agree) ∝ c_prev_normalized * exp(agree)
                    # (per-row scale from early normalization cancels) ----
                    eexp = small.tile([RPG, NG * C], F32, tag="eexp",
                                      name=f"eexp_{it}")
                    nc.scalar.activation(eexp, agree_ps, ACT.Exp)
                    if it == 0:
                        base = eexp
                    else:
                        base = small.tile([RPG, NG * C], F32, tag="esrr",
                                          name=f"base_{it}")
                        nc.vector.tensor_mul(base, esr, eexp)
                    den = small.tile([RPG, NG], F32, tag="sden",
                                     name=f"den_{it}")
                    nc.vector.reduce_sum(
                        den,
                        base.rearrange("p (g c) -> p g c", g=NG),
                        axis=mybir.AxisListType.X,
                    )
                    rec_r = small.tile([RPG, NG], F32, tag="srec",
                                       name=f"rec_{it}")
                    nc.vector.reciprocal(rec_r, den)
                    esr_n = small.tile([RPG, NG * C], F32, tag="esr",
                                       name=f"esr_{it}")
                    rec_b = rec_r.rearrange(
                        "p (g one) -> p g one", one=1
                    ).broadcast_to([RPG, NG, C])
                    nc.vector.tensor_tensor(
                        out=esr_n.rearrange("p (g c) -> p g c", g=NG),
                        in0=base.rearrange("p (g c) -> p g c", g=NG),
                        in1=rec_b, op=ALU.mult,
                    )
                    esr = esr_n       # normalized c_ij rows, [16, 90]
                elif not RDMA:
                    # ---- final iter: ReduceScatter; squash own b-shard ----
                    rs_out = dram.tile([B_SHARD * CO], CC_DT, tag="rs_out")
                    nc.gpsimd.collective_compute(
                        "ReduceScatter", ALU.add, replica_groups=rg,
                        ins=[cc_in.opt()], outs=[rs_out[:]],
                    )
                    s_sb = work.tile([B_SHARD, 1, CO], CC_DT, tag="fs")
                    nc.sync.dma_start(
                        out=s_sb,
                        in_=rs_out.rearrange("(p one n) -> p one n",
                                             n=CO, one=1),
                    )
                    t = work.tile([B_SHARD, 1, CO], F32, tag="ft")
                    bias_b1 = biasb[:B_SHARD, :].rearrange(
                        "p (one co) -> p one co", one=1
                    )
                    nc.vector.scalar_tensor_tensor(
                        out=t, in0=s_sb, scalar=1.0,
                        in1=bias_b1, op0=ALU.mult, op1=ALU.add,
                    )
                    v = _squash(nc, eps_sb, t, B_SHARD, 1, work)
                    nc.sync.dma_start(
                        out=y_d[:, :], in_=v.rearrange("p one co -> p (one co)")
                    )

    nc.compile()
    # Splice the remote-arrival gates in front of each slot reduce, now
    # that scheduling is done.  A standalone EventSemaphore on the DVE
    # queue stalls it (in-order) until the 7 remote transfers landed.
    f = nc.m.functions[0]
    for r4_name, sem in rdma_waits:
        w = nc.vector.wait_ge(sem, 14)
        w_ins = None
        for b in f.blocks:
            if b.instructions and b.instructions[-1] is w.ins:
                w_ins = b.instructions.pop()
                break
        assert w_ins is not None, "gate wait not found at a block tail"
        placed = False
        for b in f.blocks:
            for idx, i in enumerate(b.instructions):
                if i.name == r4_name:
                    b.instructions.insert(idx, w_ins)
                    placed = True
                    break
            if placed:
                break
        assert placed, f"reduce {r4_name} not found for gate splice"
    return nc


_NC = None


def kernel(x: np.ndarray, W: np.ndarray, bias: np.ndarray) -> np.ndarray:
    global _NC
    if _NC is None:
        _NC = build()

    x = np.ascontiguousarray(x, dtype=np.float32)
    W = np.ascontiguousarray(W, dtype=np.float32)
    bias = np.ascontiguousarray(bias, dtype=np.float32)

    biasf = bias.reshape(CO)
    sel = np.zeros((128, RPG), dtype=np.float32)
    sel[np.arange(128), np.arange(128) // I] = 1.0
    selT = np.ascontiguousarray(sel.T)

    in_maps = []
    for k in range(N_CORES):
        r0, r1 = k * R_LOC, (k + 1) * R_LOC
        xk = x[:, r0:r1, :].reshape(B, RI_LOC)          # [B,(r,i)]
        wk = W[r0:r1].transpose(0, 3, 1, 2).reshape(RI_LOC, CO)  # [(r,i),(c,o)]
        in_maps.append({
            "xt": np.ascontiguousarray(xk.T).astype(MM_NP),
            "xb": np.ascontiguousarray(xk).astype(MM_NP),
            "wg": np.ascontiguousarray(wk).astype(MM_NP),
            "biasf": biasf,
            "sel": sel,
            "selT": selT,
        })

    trace_cores = None
    if os.environ.get("K_TRACE_CORES"):
        trace_cores = [int(c) for c in os.environ["K_TRACE_CORES"].split(",")]

    global LAST_RESULT
    res = run_bass_kernel_spmd(
        _NC, in_maps, list(range(N_CORES)),
        trace=bool(os.environ.get("BASS_TRACE")),
        trace_cores=trace_cores,
    )
    LAST_RESULT = res
    if RDMA:
        # every core holds the full [128, NB*CO] result; b = kb*128 + p
        y0 = res.results[0]["y"].reshape(128, NB, CO)
        v = np.ascontiguousarray(y0.transpose(1, 0, 2)).reshape(B, CO)
    else:
        v = np.concatenate([res.results[k]["y"] for k in range(N_CORES)],
                           axis=0)
    return v.reshape(B, C, O)[..., None].astype(np.float32)
